# revision 34
# baseline (speedup 1.0000x reference)
"""Trainium2 Bass kernel for nn_Attention_84756884619871.

Causal multi-head attention (B=2, S=2048, D=2048, H=16, Dh=128) with RoPE,
fused QKV projection and output projection.

Sharding (8 NeuronCores): data-parallel over batch (2 groups) x
tensor-parallel over heads (4 cores/group, 4 heads each).  Each core:
  - phase A: one pass over x^T (4 chunks of 512 seq-cols): computes
    q^T,k^T (RoPE applied) for its 4 heads and v (natural layout)
  - phase B: flash-style attention in score-transposed space (p^T[t,s]);
    softmax denominator via ones-vector matmul; no max-subtraction
    (scores are small: exp is safe in fp32); normalized ctx^T shards are
    AllGather'd over the 4-core group (two halves along s so the first
    AG overlaps the second half's compute)
  - phase C: 512-column slice of the output projection (K = all heads)
All matmuls run in bf16.  Host assembles the full [2,2048,2048] output
from the 8 column slices.
"""

import numpy as np
import ml_dtypes

import concourse.bass as bass
import concourse.tile as tile
import concourse.mybir as mybir
from concourse import bacc
from concourse import bass_isa
from contextlib import ExitStack

F32 = mybir.dt.float32
F32R = mybir.dt.bfloat16
AF = mybir.ActivationFunctionType

D = 2048
S = 2048
NCORES = 8
TPDEG = 4          # tensor-parallel group size (heads)
HLOC = 4           # heads per core
DH = 128
SCALE = float(1.0 / np.sqrt(DH))

_STATE: dict = {}


def _chunks(c0):
    """Split columns [c0, 1024) into <=512 pieces."""
    if c0 == 0:
        return [(0, 512), (512, 1024)]
    if c0 < 512:
        return [(c0, 512), (512, 1024)]
    if c0 < 1024:
        return [(c0, 1024)]
    raise ValueError(c0)


def _build():
    nc = bacc.Bacc("TRN2", target_bir_lowering=False, debug=False, num_devices=NCORES)
    xT = nc.dram_tensor("xT", [D, S], F32R, kind="ExternalInput")
    wqk = nc.dram_tensor("wqk", [D, 1024], F32R, kind="ExternalInput")
    wv = nc.dram_tensor("wv", [D, 512], F32R, kind="ExternalInput")
    wo = nc.dram_tensor("wo", [D, 512], F32R, kind="ExternalInput")
    cosT = nc.dram_tensor("cosT", [128, S], F32, kind="ExternalInput")
    sinTs = nc.dram_tensor("sinTs", [128, S], F32, kind="ExternalInput")
    trimask = nc.dram_tensor("trimask", [128, 128], F32R, kind="ExternalInput")
    out = nc.dram_tensor("out", [S, 512], F32, kind="ExternalOutput")

    xT3 = xT.ap().rearrange("(ko ki) s -> ki ko s", ki=128)
    wqk3 = wqk.ap().rearrange("(ko ki) c -> ki ko c", ki=128)
    wv3 = wv.ap().rearrange("(ko ki) c -> ki ko c", ki=128)
    wo3 = wo.ap().rearrange("(ko ki) c -> ki ko c", ki=128)

    # AllGather scratch: 4 segments of 512 query columns
    NSEG = 4
    agin = [nc.dram_tensor(f"agin{sg}", [HLOC * 128, 512], F32R)
            for sg in range(NSEG)]
    agout = [nc.dram_tensor(f"agout{sg}", [D, 512], F32R)
             for sg in range(NSEG)]
    agout3 = [t.ap().rearrange("(ko ki) c -> ki ko c", ki=128) for t in agout]

    with tile.TileContext(nc) as tc, ExitStack() as top:
        # ---- persistent tensors -------------------------------------------
        per = top.enter_context(tc.tile_pool(name="persist", bufs=1))
        qk_pool = top.enter_context(tc.tile_pool(name="qkpool", bufs=1))
        qrot = [qk_pool.tile([128, S], F32R, name=f"qrot{h}") for h in range(HLOC)]
        krot = [qk_pool.tile([128, S], F32R, name=f"krot{h}") for h in range(HLOC)]
        v_pool = top.enter_context(tc.tile_pool(name="vpool", bufs=1))
        vsb = [v_pool.tile([128, 512], F32R, name=f"v{j}") for j in range(16)]
        aw = top.enter_context(ExitStack())  # pools freed after phase A
        w_pool = aw.enter_context(tc.tile_pool(name="wpool", bufs=1))
        # v projection runs first, so wv + x chunk 0 (4MB) gate the first
        # matmul; wqk (4MB) lands during the v matmuls
        wv_sb = [w_pool.tile([128, 4, 512], F32R, name=f"wv{g}") for g in range(4)]
        for g in range(4):
            nc.sync.dma_start(wv_sb[g][:], wv3[:, 4 * g:4 * g + 4, :])
        xt_pool = aw.enter_context(tc.tile_pool(name="xt1", bufs=2))
        xt_tiles = {}
        xt_tiles[0] = xt_pool.tile([128, 16, 512], F32R, tag="xt", name="xt_0")
        # first chunk split in two so the first v matmuls start sooner
        nc.sync.dma_start(xt_tiles[0][:, 0:8, :], xT3[:, 0:8, 0:512])
        nc.sync.dma_start(xt_tiles[0][:, 8:16, :], xT3[:, 8:16, 0:512])
        wqk_sb = [w_pool.tile([128, 4, 1024], F32R, name=f"wqk{g}") for g in range(4)]
        for g in range(4):
            nc.sync.dma_start(wqk_sb[g][:], wqk3[:, 4 * g:4 * g + 4, :])
        cs_pool = aw.enter_context(tc.tile_pool(name="csp", bufs=1))
        cos_sb = cs_pool.tile([128, S], F32, name="cos_sb")
        sin_sb = cs_pool.tile([128, S], F32, name="sin_sb")
        nc.sync.dma_start(cos_sb[:], cosT.ap())
        nc.sync.dma_start(sin_sb[:], sinTs.ap())
        mask_sb = per.tile([128, 128], F32R, name="mask")
        nc.sync.dma_start(mask_sb[:], trimask.ap())

        # ---- phase A: fused q^T,k^T (RoPE) + v projection -----------------
        with ExitStack() as st, nc.named_scope("phaseA"):
            tmp_pool = st.enter_context(tc.tile_pool(name="ropetmp", bufs=4))
            ps_qk = st.enter_context(tc.tile_pool(name="psqk", bufs=2, space="PSUM"))
            ps_v = st.enter_context(tc.tile_pool(name="psv", bufs=2, space="PSUM"))
            for sc in range(4):
                if sc in xt_tiles:
                    xt_c = xt_tiles[sc]
                else:
                    xt_c = xt_pool.tile([128, 16, 512], F32R, tag="xt",
                                        name=f"xt_{sc}")
                    nc.sync.dma_start(xt_c[:], xT3[:, :, 512 * sc:512 * sc + 512])
                sl = slice(512 * sc, 512 * sc + 512)
                for u in range(4):
                    j = 4 * sc + u
                    pv = ps_v.tile([128, 512], F32, tag="psv", name=f"pv{j}")
                    for g in range(4):
                        for ki in range(4):
                            ko = 4 * g + ki
                            nc.tensor.matmul(
                                pv[:], xt_c[:, ko, 128 * u:128 * u + 128],
                                wv_sb[g][:, ki, :], start=(ko == 0), stop=(ko == 15),
                            )
                    nc.vector.tensor_copy(vsb[j][:], pv[:])
                for m in range(8):
                    pq = ps_qk.tile([128, 512], F32, tag="psqk", name=f"pq{sc}_{m}")
                    for g in range(4):
                        for ki in range(4):
                            ko = 4 * g + ki
                            nc.tensor.matmul(
                                pq[:], wqk_sb[g][:, ki, 128 * m:128 * m + 128],
                                xt_c[:, ko, :], start=(ko == 0), stop=(ko == 15),
                            )
                    dest = (qrot[m] if m < 4 else krot[m - 4])[:, sl]
                    t1 = tmp_pool.tile([128, 512], F32, tag="t1", name=f"t1_{sc}_{m}")
                    nc.vector.tensor_mul(t1[:], pq[:], cos_sb[:, sl])
                    t2 = tmp_pool.tile([128, 512], F32, tag="t2", name=f"t2_{sc}_{m}")
                    nc.vector.tensor_mul(t2[0:64, :], pq[64:128, :], sin_sb[0:64, sl])
                    nc.vector.tensor_mul(t2[64:128, :], pq[0:64, :], sin_sb[64:128, sl])
                    nc.vector.tensor_add(dest, t1[:], t2[:])

        aw.close()  # free wqk/wv/cos/sin SBUF before ctxg allocation

        # wo prefetch (consumed in phase C; DMA overlaps phase B)
        wo_pool = top.enter_context(tc.tile_pool(name="wop", bufs=1))
        wo_sb = wo_pool.tile([128, 16, 512], F32R, name="wo_sb")
        nc.sync.dma_start(wo_sb[:], wo3[:])

        # ---- phase B: attention (score-transposed flash), 4 segments ------
        cg_pool = top.enter_context(tc.tile_pool(name="cgp", bufs=1))
        ctxg = [cg_pool.tile([128, 16, 512], F32R, name=f"ctxg{sg}")
                for sg in range(NSEG)]
        osb_pool = top.enter_context(tc.tile_pool(name="osbp", bufs=3))

        def emit_out_block(m, po_pool):
            """One 128-row slab of the output projection from gathered ctx."""
            sg, mm = m // 4, m % 4
            po = po_pool.tile([128, 512], F32, tag="po", name=f"po{m}")
            for ko in range(16):
                nc.tensor.matmul(
                    po[:], ctxg[sg][:, ko, 128 * mm:128 * mm + 128],
                    wo_sb[:, ko, :], start=(ko == 0), stop=(ko == 15),
                )
            osb = osb_pool.tile([128, 512], F32, tag="osb", name=f"osb{m}")
            nc.vector.tensor_copy(osb[:], po[:])
            nc.sync.dma_start(out.ap()[128 * m:128 * m + 128, :], osb[:])
        with ExitStack() as st, nc.named_scope("phaseB"):
            p_pool = st.enter_context(tc.tile_pool(name="pp", bufs=4))
            misc = st.enter_context(tc.tile_pool(name="miscb", bufs=4))
            pa_pool = st.enter_context(tc.tile_pool(name="pacc", bufs=3))
            sc_ps = st.enter_context(tc.tile_pool(name="scps", bufs=2, space="PSUM"))
            ctx_ps = st.enter_context(tc.tile_pool(name="ctxps", bufs=4, space="PSUM"))
            for sg in range(NSEG):
                q0 = 512 * sg                     # segment query offset
                jmax = 4 * sg + 4
                for h in range(HLOC):
                    ctx = ctx_ps.tile([128, 512], F32, tag="ctx",
                                      name=f"ctx{sg}_{h}")
                    # softmax denominator: per-partition partial sums of p
                    # accumulate on DVE; GpSimd does the cross-partition
                    # reduce at the end (result arrives pre-broadcast)
                    pacc = pa_pool.tile([128, 512], F32R, tag="pa",
                                        name=f"pa{sg}_{h}")
                    # key blocks processed in pairs sharing one [128,1024]
                    # score tile: a single exp covers both halves (the stale
                    # gap between diagonal chunks is exp'd but never read)
                    for ja in range(0, jmax, 2):
                        jb = ja + 1
                        c0a = max(0, 128 * (ja - 4 * sg))
                        c0b = max(0, 128 * (jb - 4 * sg))
                        sc2 = sc_ps.tile([128, 1024], F32, tag="scps",
                                         name=f"sc{sg}_{h}_{ja}")
                        nc.tensor.matmul(
                            sc2[:, c0a:512], krot[h][:, 128 * ja:128 * ja + 128],
                            qrot[h][:, q0 + c0a:q0 + 512],
                            start=True, stop=True,
                        )
                        nc.tensor.matmul(
                            sc2[:, 512 + c0b:1024],
                            krot[h][:, 128 * jb:128 * jb + 128],
                            qrot[h][:, q0 + c0b:q0 + 512],
                            start=True, stop=True,
                        )
                        p2 = p_pool.tile([128, 1024], F32R, tag="p",
                                         name=f"p{sg}_{h}_{ja}")
                        nc.scalar.activation(p2[:, c0a:1024], sc2[:, c0a:1024],
                                             AF.Exp, scale=SCALE)
                        if ja >= 4 * sg:
                            nc.vector.tensor_mul(p2[:, c0a:c0a + 128],
                                                 p2[:, c0a:c0a + 128], mask_sb[:])
                        if jb >= 4 * sg:
                            o = 512 + c0b
                            nc.vector.tensor_mul(p2[:, o:o + 128],
                                                 p2[:, o:o + 128], mask_sb[:])
                        last = (jb == jmax - 1)
                        nc.tensor.matmul(
                            ctx[:, c0a:512], vsb[ja][:, 128 * h:128 * h + 128],
                            p2[:, c0a:512], start=(ja == 0), stop=False,
                            skip_group_check=True,
                        )
                        nc.tensor.matmul(
                            ctx[:, c0b:512], vsb[jb][:, 128 * h:128 * h + 128],
                            p2[:, 512 + c0b:1024], start=(jb == 0), stop=last,
                            skip_group_check=True,
                        )
                        if ja == 0:
                            nc.vector.tensor_copy(pacc[:], p2[:, 0:512])
                        else:
                            nc.vector.tensor_add(pacc[:, c0a:512],
                                                 pacc[:, c0a:512], p2[:, c0a:512])
                        nc.vector.tensor_add(pacc[:, c0b:512], pacc[:, c0b:512],
                                             p2[:, 512 + c0b:1024])
                    # normalize: ctxn = ctx * (1/l); l arrives broadcast over
                    # all partitions from the GpSimd all-reduce
                    l_bc = misc.tile([128, 512], F32, tag="lbc", name=f"lb{sg}_{h}")
                    nc.gpsimd.partition_all_reduce(
                        l_bc[:], pacc[:], channels=128,
                        reduce_op=bass_isa.ReduceOp.add,
                    )
                    linv_bc = misc.tile([128, 512], F32, tag="linvbc",
                                        name=f"lv{sg}_{h}")
                    nc.vector.reciprocal_approx_fast(out=linv_bc[:], in_=l_bc[:])
                    ctxn = misc.tile([128, 512], F32R, tag="ctxn", name=f"cn{sg}_{h}")
                    nc.vector.tensor_mul(ctxn[:], ctx[:], linv_bc[:])
                    nc.sync.dma_start(
                        agin[sg].ap()[128 * h:128 * h + 128, :], ctxn[:]
                    )
                nc.gpsimd.collective_compute(
                    "AllGather", mybir.AluOpType.bypass,
                    ins=[agin[sg].ap()], outs=[agout[sg].ap()],
                    replica_groups=[[0, 1, 2, 3], [4, 5, 6, 7]],
                )
                # gathered ctx^T load, in ko-groups so phase C matmuls can
                # start as soon as the first group lands
                for g in range(4):
                    nc.sync.dma_start(ctxg[sg][:, 4 * g:4 * g + 4, :],
                                      agout3[sg][:, 4 * g:4 * g + 4, :])

        # ---- phase C: output projection (512-col slice, K = all heads) ----
        with ExitStack() as st, nc.named_scope("phaseC"):
            ps_o = st.enter_context(tc.tile_pool(name="pso", bufs=4, space="PSUM"))
            for m in range(16):
                emit_out_block(m, ps_o)

    nc.compile()
    return nc


def _get_runner():
    """Build (once) a persistent jitted SPMD executor for the kernel program."""
    if "runner" in _STATE:
        return _STATE["runner"]
    import jax
    from jax.sharding import Mesh, PartitionSpec
    from jax.experimental.shard_map import shard_map
    from concourse import bass2jax

    nc = _build()
    bass2jax.install_neuronx_cc_hook()

    in_names, out_names, out_avals = [], [], []
    for alloc in nc.m.functions[0].allocations:
        if not isinstance(alloc, mybir.MemoryLocationSet):
            continue
        name = alloc.memorylocations[0].name
        pname = nc.partition_id_tensor.name if nc.partition_id_tensor else None
        if alloc.kind == "ExternalInput":
            if name != pname:
                in_names.append(name)
        elif alloc.kind == "ExternalOutput":
            out_names.append(name)
            out_avals.append(
                jax.core.ShapedArray(tuple(alloc.tensor_shape),
                                     mybir.dt.np(alloc.dtype))
            )
    n_params = len(in_names)
    all_in = list(in_names) + list(out_names)
    pname = nc.partition_id_tensor.name if nc.partition_id_tensor else None
    if pname is not None:
        all_in.append(pname)

    def _body(*args):
        operands = list(args)
        if pname is not None:
            operands.append(bass2jax.partition_id_tensor())
        outs = bass2jax._bass_exec_p.bind(
            *operands,
            out_avals=tuple(out_avals),
            in_names=tuple(all_in),
            out_names=tuple(out_names),
            lowering_input_output_aliases=(),
            sim_require_finite=False,
            sim_require_nnan=False,
            nc=nc,
        )
        return tuple(outs)

    devices = jax.devices()[:NCORES]
    mesh = Mesh(np.asarray(devices), ("core",))
    specs = (PartitionSpec("core"),)
    sharded = jax.jit(
        shard_map(
            _body, mesh=mesh,
            in_specs=specs * (n_params + len(out_names)),
            out_specs=specs * len(out_names),
            check_rep=False,
        ),
        keep_unused=True,
    )
    runner = {
        "fn": sharded, "in_names": in_names, "out_names": out_names,
        "out_avals": out_avals, "n_params": n_params,
    }
    _STATE["runner"] = runner
    return runner


def _prep_inputs(x, cos, sin, w_qkv, w_o):
    """Host-side sharding: per-core input dict list."""
    x = np.asarray(x, dtype=np.float32)
    cos = np.asarray(cos, dtype=np.float32)
    sin = np.asarray(sin, dtype=np.float32)
    w_qkv = np.asarray(w_qkv, dtype=np.float32)
    w_o = np.asarray(w_o, dtype=np.float32)

    cosT = np.ascontiguousarray(cos.T)                      # [128, S]
    sinT = sin.T
    sinTs = np.ascontiguousarray(
        np.concatenate([-sinT[0:64], sinT[64:128]], axis=0))
    pp, ff = np.meshgrid(np.arange(128), np.arange(128), indexing="ij")
    trimask = (pp <= ff).astype(np.float32)                 # t <= s

    in_maps = []
    for c in range(NCORES):
        b, tp = c // TPDEG, c % TPDEG
        cs = 512 * tp
        xT = np.ascontiguousarray(x[b].T)                   # [D, S]
        wq = w_qkv[:, cs:cs + 512]
        wk = w_qkv[:, D + cs:D + cs + 512]
        wqk = np.ascontiguousarray(np.concatenate([wq, wk], axis=1))
        wvs = np.ascontiguousarray(w_qkv[:, 2 * D + cs:2 * D + cs + 512])
        wos = np.ascontiguousarray(w_o[:, cs:cs + 512])
        bf = ml_dtypes.bfloat16
        in_maps.append({
            "xT": xT.astype(bf), "wqk": wqk.astype(bf), "wv": wvs.astype(bf),
            "wo": wos.astype(bf),
            "cosT": cosT, "sinTs": sinTs, "trimask": trimask.astype(bf),
        })
    return in_maps


def _run(in_maps):
    import jax
    r = _get_runner()
    concat = [
        np.concatenate([np.asarray(in_maps[c][n]) for c in range(NCORES)], axis=0)
        for n in r["in_names"]
    ]
    zeros = [
        np.zeros((NCORES * a.shape[0],) + tuple(a.shape[1:]), a.dtype)
        for a in r["out_avals"]
    ]
    outs = r["fn"](*concat, *zeros)
    outs = [np.asarray(o) for o in jax.block_until_ready(outs)]
    per_core = []
    for c in range(NCORES):
        d = {}
        for i, n in enumerate(r["out_names"]):
            shp = r["out_avals"][i].shape
            d[n] = outs[i].reshape((NCORES,) + shp)[c]
        per_core.append(d)
    return per_core


def kernel(x, cos, sin, w_qkv, w_o):
    in_maps = _prep_inputs(x, cos, sin, w_qkv, w_o)
    results = _run(in_maps)
    B = x.shape[0]
    out = np.empty((B, S, D), dtype=np.float32)
    for c in range(NCORES):
        b, tp = c // TPDEG, c % TPDEG
        out[b, :, 512 * tp:512 * tp + 512] = results[c]["out"]
    return out
